# revision 34
# baseline (speedup 1.0000x reference)
"""MoE top-2 routed linear (nn_MoELinear) on 8 Trainium2 NeuronCores.

Strategy (expert parallelism, per the sharding hint):
  - Gating (tiny: [N,1024]x[1024,8] matmul + top-2 + softmax) is computed on
    host with jax-CPU, replicating the reference op-for-op so the top-2
    decisions match the reference bitwise.
  - Tokens are dispatched (gathered) per expert on host; core e receives the
    tokens routed to expert e (padded to a common capacity C), expert e's
    weights pre-transposed to [d_in, d_out], and the per-token gate weight.
  - Each core computes Y_e = (X_e @ We[e].T) * w_e[:, None]  -- a dense
    [C,1024]x[1024,4096] matmul with the gate scale applied on-chip during
    PSUM eviction.  Expert weights are cached entirely in SBUF.
  - Host combines: out[token] = sum of its (two) expert contributions.
"""

import os

import numpy as np

NUM_CORES = 8
TOP_K = 2
P = 128  # partitions
N_TILE = 512  # psum free-dim tile (one bank of fp32)

# matmul dtype knob: "float32" (exact, 4 cyc/row), "float32r" (full rate,
# ~1.5e-4 rel err), "bfloat16" (full rate, halves input DMA, ~3e-3 rel err)
MM_DTYPE = os.environ.get("MOE_MM_DTYPE", "float32r")
# enable NTFF tracing (sets LAST_RUN_INFO["exec_time_ns"])
TRACE = os.environ.get("MOE_TRACE", "0") == "1"

LAST_RUN_INFO = {}
_NC_CACHE = {}


def _routing(x_flat, Wg, bg):
    """Replicate the reference gating bitwise on jax-CPU; numpy fallback."""
    try:
        import jax
        import jax.numpy as jnp

        with jax.default_device(jax.devices("cpu")[0]):
            xf = jnp.asarray(x_flat)
            gate_logits = xf @ jnp.asarray(Wg).T + jnp.asarray(bg)
            top_w, top_idx = jax.lax.top_k(gate_logits, TOP_K)
            top_w = jax.nn.softmax(top_w, axis=-1)
            return np.asarray(top_idx), np.asarray(top_w)
    except Exception:
        logits = x_flat @ Wg.T + bg
        top_idx = np.argsort(-logits, axis=1, kind="stable")[:, :TOP_K]
        top_v = np.take_along_axis(logits, top_idx, axis=1)
        e = np.exp(top_v - top_v.max(axis=1, keepdims=True))
        top_w = e / e.sum(axis=1, keepdims=True)
        return top_idx, top_w.astype(np.float32)


def _build_program(C, CIN, DOUT, mm_dtype):
    """One-expert program: y[C,DOUT] = (xt[CIN,C].T @ wt[CIN,DOUT]) * sc."""
    import concourse.mybir as mybir
    import concourse.tile as tile
    from concourse import bacc

    f32 = mybir.dt.float32
    if mm_dtype == "bfloat16":
        io_dt = mybir.dt.bfloat16
        mm_dt = mybir.dt.bfloat16
    elif mm_dtype == "float32r":
        io_dt = mybir.dt.float32r
        mm_dt = mybir.dt.float32r
    else:
        io_dt = f32
        mm_dt = f32

    KT = CIN // P
    MT = C // P
    NT = DOUT // N_TILE

    nc = bacc.Bacc()
    # x pre-tiled on host: xt[m, p, kt*128+j] = token (m*128+j), cin (kt*128+p)
    xt = nc.declare_dram_parameter("xt", [C // P, P, CIN], io_dt, isOutput=False)
    wt = nc.declare_dram_parameter("wt", [CIN, DOUT], io_dt, isOutput=False)
    # scales pre-transposed on host: sc[p, m] = gate weight of token m*128+p
    sc = nc.declare_dram_parameter("sc", [P, C // P], f32, isOutput=False)
    y = nc.declare_dram_parameter("y", [C, DOUT], f32, isOutput=True)

    # Split the n range into halves.  W is DMA'd n-half-major so the first
    # half's output groups become fully computable after only half the W
    # load; x tiles are (re)loaded once per half.  This hides the 47us W
    # load behind matmuls instead of stalling the PE at kernel start.
    NH = 2 if NT % 2 == 0 and MT > 1 else 1
    NTH = NT // NH  # n-tiles per half
    WH = NTH * N_TILE  # output columns per half

    PF = 4  # x-tile prefetch depth of the per-half software pipeline
    XBUFS = min(9, MT)

    with tile.TileContext(nc) as tc:
        with (
            tc.tile_pool(name="wpool", bufs=1) as wpool,
            tc.tile_pool(name="xpool", bufs=XBUFS) as xpool,
            tc.tile_pool(name="spool", bufs=1) as spool,
            tc.tile_pool(name="opool", bufs=6) as opool,
            tc.tile_pool(name="pspool", bufs=8, space="PSUM") as pspool,
        ):
            # DMA queue plan: W owns the sync HWDGE queues; half-0 x tiles,
            # scales, evictions and y stores ride the scalar HWDGE queues;
            # half-1 x tiles go back on sync (idle once W is resident).
            # One 3D DMA per x tile keeps trigger counts low.

            def load_xm(m, eng):
                # lhsT tiles for token block m: [P(cin chunk), P(tokens)] x KT,
                # host-pretiled so this is one fully-contiguous 2D DMA
                xtile = xpool.tile([P, KT * P], io_dt, name="xtile", tag="xtile")
                eng.dma_start(out=xtile[:], in_=xt[m])
                return xtile

            # all scales in one contiguous DMA (tiny; ahead of the W load)
            sctile = spool.tile([P, MT], f32)
            nc.sync.dma_start(out=sctile[:], in_=sc[:, :])
            stiles = [sctile[:, m : m + 1] for m in range(MT)]

            wtiles = [
                wpool.tile([P, DOUT], io_dt, tag=f"w{k}", name=f"w{k}")
                for k in range(KT)
            ]

            def load_w_cols(c0, c1):
                for k in range(KT):
                    nc.sync.dma_start(
                        out=wtiles[k][:, c0:c1],
                        in_=wt[k * P : (k + 1) * P, c0:c1],
                    )

            # W arrives in n-major pieces, finest first, so the earliest
            # output groups become fully computable after only a sliver of
            # the W load instead of stalling the PE behind the whole 17MB
            for c0 in range(0, WH, N_TILE):
                load_w_cols(c0, c0 + N_TILE)
            for nh in range(1, NH):
                load_w_cols(nh * WH, (nh + 1) * WH)

            pfe = min(PF, MT)
            next_pending = []
            for nh in range(NH):
                eng = nc.scalar if nh == 0 else nc.sync
                if nh == 0:
                    pending = [load_xm(m, eng) for m in range(pfe)]
                else:
                    pending = next_pending
                for m in range(MT):
                    xtile = pending.pop(0)
                    if m + pfe < MT:
                        pending.append(load_xm(m + pfe, eng))
                    stile = stiles[m]

                    for n in range(nh * NTH, (nh + 1) * NTH):
                        psum = pspool.tile([P, N_TILE], f32)
                        for k in range(KT):
                            nc.tensor.matmul(
                                psum[:],
                                lhsT=xtile[:, k * P : (k + 1) * P].bitcast(mm_dt),
                                rhs=wtiles[k][
                                    :, n * N_TILE : (n + 1) * N_TILE
                                ].bitcast(mm_dt),
                                start=(k == 0),
                                stop=(k == KT - 1),
                            )
                        otile = opool.tile([P, N_TILE], f32)
                        nc.scalar.activation(
                            otile[:],
                            psum[:],
                            mybir.ActivationFunctionType.Copy,
                            scale=stile[:],
                        )
                        nc.scalar.dma_start(
                            out=y[m * P : (m + 1) * P, n * N_TILE : (n + 1) * N_TILE],
                            in_=otile[:],
                        )

                    # prefetch next half's first x tiles across the boundary
                    if nh == 0 and NH > 1 and m >= MT - pfe:
                        next_pending.append(load_xm(m - (MT - pfe), nc.sync))
    nc.finalize()
    return nc


def kernel(x, We, Wg, bg):
    from concourse.bass_utils import run_bass_kernel_spmd

    B, T, CIN = x.shape
    E, DOUT, _ = We.shape
    N = B * T
    x_flat = np.ascontiguousarray(x.reshape(N, CIN), dtype=np.float32)

    top_idx, top_w = _routing(x_flat, Wg, bg)

    # dispatch: token lists per expert
    idx_e = []
    w_e = []
    for e in range(E):
        sel0 = top_idx[:, 0] == e
        sel1 = top_idx[:, 1] == e
        rows = np.nonzero(sel0 | sel1)[0]
        w = np.where(sel0[rows], top_w[rows, 0], top_w[rows, 1]).astype(np.float32)
        idx_e.append(rows)
        w_e.append(w)

    cmax = max(len(r) for r in idx_e)
    C = max(P, ((cmax + P - 1) // P) * P)

    io_np = np.float32
    if MM_DTYPE == "bfloat16":
        import ml_dtypes

        io_np = ml_dtypes.bfloat16

    in_maps = []
    for e in range(E):
        ce = len(idx_e[e])
        xg = np.zeros((C, CIN), np.float32)
        xg[:ce] = x_flat[idx_e[e]]
        # pre-tile for the device: [m-tile, cin-within-chunk, kt*128 + token]
        xt = np.ascontiguousarray(
            xg.reshape(C // 128, 128, CIN // 128, 128).transpose(0, 3, 2, 1)
        ).reshape(C // 128, 128, CIN).astype(io_np)
        wt = np.ascontiguousarray(We[e].T).astype(io_np)
        scf = np.zeros(C, np.float32)
        scf[:ce] = w_e[e]
        sc = np.ascontiguousarray(scf.reshape(C // 128, 128).T)  # [P, MT]
        in_maps.append({"xt": xt, "wt": wt, "sc": sc})

    key = (C, CIN, DOUT, MM_DTYPE)
    if key not in _NC_CACHE:
        _NC_CACHE[key] = _build_program(C, CIN, DOUT, MM_DTYPE)
    nc = _NC_CACHE[key]
    res = run_bass_kernel_spmd(nc, in_maps, list(range(NUM_CORES)), trace=TRACE)

    LAST_RUN_INFO.clear()
    LAST_RUN_INFO.update(
        exec_time_ns=res.exec_time_ns,
        mean_exec_time_ns=res.mean_exec_time_ns,
        max_exec_time_core_id=res.max_exec_time_core_id,
        profile_json=res.profile_json,
    )

    out = np.zeros((N, DOUT), np.float32)
    for e in range(E):
        ye = res.results[e]["y"]
        out[idx_e[e]] += ye[: len(idx_e[e])]
    return out.reshape(B, T, DOUT)
